# revision 14
# baseline (speedup 1.0000x reference)
"""Trainium2 Bass kernel for nn_Encoder_meta (GRU with per-step meta transform).

Reference computation (per batch row b, over T steps):
    x_cat = concat(x_l, x_t, x_w, x_s)            # [B, T, 160]
    x' = tanh(h @ Wt_h.T + x_t @ Wt_x.T + bt)     # [B, 256]
    gx = x' @ W_ih.T + b_ih ; gh = h @ W_hh.T + b_hh
    r = sig(gxr+ghr); z = sig(gxz+ghz); n = tanh(gxn + r*ghn)
    h = (1-z)*n + z*h
Output: h_T as [1, B, H].

Strategy: data-parallel over batch across 8 cores (32 rows each).
Precompute u[t] = x_cat[t] @ Wt_x.T + bt for all t with fp32r matmuls
(time-batched, tensor-engine friendly), keep u in SBUF. The serial
recurrence runs with bf16 matmuls in a col-tiled "batch-stationary"
layout: all per-step tensors live as [128, F] tiles where partition
p = 32*g + b encodes (hidden-block g, batch row b), so elementwise gate
math uses all 128 partitions, and the 4 hidden blocks map to the 4
PE-array column groups (concurrent matmuls via tile_position).
"""

import os
from contextlib import ExitStack

import numpy as np
import ml_dtypes

import concourse.bass as bass
import concourse.mybir as mybir
import concourse.tile as tile
from concourse import bacc
from concourse.bass_utils import run_bass_kernel_spmd

F32 = mybir.dt.float32
F32R = mybir.dt.float32r
BF16 = mybir.dt.bfloat16
BF = ml_dtypes.bfloat16

B, T, H, XP, D = 256, 512, 512, 256, 160
DA = D + 1  # ones row folded in for bt
NCORES, BC = 8, 32  # cores, batch per core
GATE3 = 3 * H  # 1536

Act = mybir.ActivationFunctionType


def _build_nc():
    nc = bacc.Bacc("TRN2", target_bir_lowering=False, debug=False, num_devices=NCORES)

    xT = nc.dram_tensor("xT", [DA, T * BC], F32R, kind="ExternalInput")
    wtx = nc.dram_tensor("wtx", [DA, XP], F32R, kind="ExternalInput")
    wthT = nc.dram_tensor("wthT", [128, 4 * XP], BF16, kind="ExternalInput")
    wih = nc.dram_tensor("wih", [128, 2 * GATE3], BF16, kind="ExternalInput")
    whh = nc.dram_tensor("whh", [128, 4 * GATE3], BF16, kind="ExternalInput")
    ball = nc.dram_tensor("ball", [128, 512], F32, kind="ExternalInput")
    hout = nc.dram_tensor("hout", [128, 128], F32, kind="ExternalOutput")

    with tile.TileContext(nc) as tc:
        _kernel_body(tc, xT, wtx, wthT, wih, whh, ball, hout)
    nc.compile()
    return nc


def _kernel_body(tc, xT, wtx, wthT, wih, whh, ball, hout):
    nc = tc.nc
    with ExitStack() as ctx:
        const = ctx.enter_context(tc.tile_pool(name="const", bufs=1))

        wthT_sb = const.tile([128, 4 * XP], BF16)
        nc.sync.dma_start(out=wthT_sb[:], in_=wthT.ap())
        wih_sb = const.tile([128, 2 * GATE3], BF16)
        nc.sync.dma_start(out=wih_sb[:], in_=wih.ap())
        whh_sb = const.tile([128, 4 * GATE3], BF16)
        nc.sync.dma_start(out=whh_sb[:], in_=whh.ap())
        ball_sb = const.tile([128, 512], F32)
        nc.sync.dma_start(out=ball_sb[:], in_=ball.ap())
        wtx0_sb = const.tile([128, XP], F32R)
        nc.sync.dma_start(out=wtx0_sb[:], in_=wtx.ap()[0:128, :])
        wtx1_sb = const.tile([DA - 128, XP], F32R)
        nc.sync.dma_start(out=wtx1_sb[:], in_=wtx.ap()[128:DA, :])

        ident = const.tile([128, 128], F32)
        from concourse.masks import make_identity

        make_identity(nc, ident[:])

        # u[t] in xp-row-transposed layout: [p, t, mt*32 + b], p = xp row % 128
        u_sb = const.tile([128, T, 64], F32)
        h_sb = const.tile([128, 128], F32)  # h[32g+b, f] = h[b, 128g+f]
        hT_sb = const.tile([128, 128], BF16)  # hT[p, 32kt+b] = h[b, 128kt+p]
        nc.vector.memset(h_sb[:], 0.0)
        nc.vector.memset(hT_sb[:], 0.0)

        # ---- precompute u = [Wt_x | bt] @ [x; 1]  (fp32r, big-N) ----
        xpool = ctx.enter_context(tc.tile_pool(name="xt", bufs=3))
        ups = ctx.enter_context(tc.tile_pool(name="ups", bufs=2, space="PSUM"))
        CW = 512  # (t,b) pairs per fp32r matmul chunk
        while (T * BC) % CW:
            CW //= 2
        NCH = (T * BC) // CW
        SPC = CW // BC  # timesteps per chunk
        for c in range(NCH):
            xt0 = xpool.tile([128, CW], F32R, tag="xt0")
            nc.sync.dma_start(out=xt0[:], in_=xT.ap()[0:128, CW * c : CW * (c + 1)])
            xt1 = xpool.tile([DA - 128, CW], F32R, tag="xt1")
            nc.sync.dma_start(out=xt1[:], in_=xT.ap()[128:DA, CW * c : CW * (c + 1)])
            for mt in range(2):
                up = ups.tile([128, SPC, 32], F32)
                nc.tensor.matmul(
                    up[:],
                    wtx0_sb[:, 128 * mt : 128 * (mt + 1)],
                    xt0[:],
                    start=True,
                    stop=False,
                )
                nc.tensor.matmul(
                    up[:],
                    wtx1_sb[:, 128 * mt : 128 * (mt + 1)],
                    xt1[:],
                    start=False,
                    stop=True,
                )
                nc.vector.tensor_copy(
                    u_sb[:, SPC * c : SPC * (c + 1), 32 * mt : 32 * (mt + 1)], up[:]
                )

        # ---- recurrence ----
        gps = ctx.enter_context(tc.tile_pool(name="gps", bufs=2, space="PSUM"))
        xps = ctx.enter_context(tc.tile_pool(name="xps", bufs=2, space="PSUM"))
        tps = ctx.enter_context(tc.tile_pool(name="tps", bufs=2, space="PSUM"))
        work = ctx.enter_context(tc.tile_pool(name="work", bufs=2))

        for t in range(T):
            # x' = tanh(Wt_h @ h + u_t): weights-stationary, out [xp-row, b]
            # NOTE: a matmul with start=True marks the output partitions'
            # whole 2KB psum bank row "pending-zero", so each (partition
            # range, region) accumulation group must run start..stop
            # strictly sequentially before any other start touches those
            # partitions.
            xp_ps = xps.tile([128, 64], F32)
            for mt in range(2):
                o = xp_ps[:, 32 * mt : 32 * (mt + 1)]
                for kt in range(4):
                    nc.tensor.matmul(
                        o,
                        wthT_sb[:, XP * kt + 128 * mt : XP * kt + 128 * (mt + 1)],
                        hT_sb[:, 32 * kt : 32 * (kt + 1)],
                        start=(kt == 0),
                        stop=(kt == 3),
                    )
            xp_s = work.tile([128, 64], F32, tag="xp_s")
            nc.vector.tensor_add(xp_s[:], xp_ps[:], u_sb[:, t, :])
            xp_bf = work.tile([128, 64], BF16, tag="xp_bf")
            nc.scalar.activation(xp_bf[:], xp_s[:], Act.Tanh)

            # gates psum [128, 512]: segs r|z|gxn|ghn, partition 32g+b.
            # Each (g, seg) region is one complete start..stop group; groups
            # in different col-groups interleave for PE-array concurrency.
            g_ps = gps.tile([128, 512], F32)

            def seg(g, s):
                return g_ps[32 * g : 32 * (g + 1), 128 * s : 128 * (s + 1)]

            def hT_mm(g, s, gcol, kt, start, stop):
                nc.tensor.matmul(
                    seg(g, s),
                    hT_sb[:, 32 * kt : 32 * (kt + 1)],
                    whh_sb[:, GATE3 * kt + 512 * gcol + 128 * g :
                           GATE3 * kt + 512 * gcol + 128 * (g + 1)],
                    start=start, stop=stop, tile_position=(0, 32 * g),
                )

            def xp_mm(g, s, gcol, kt, start, stop):
                nc.tensor.matmul(
                    seg(g, s),
                    xp_bf[:, 32 * kt : 32 * (kt + 1)],
                    wih_sb[:, GATE3 * kt + 512 * gcol + 128 * g :
                           GATE3 * kt + 512 * gcol + 128 * (g + 1)],
                    start=start, stop=stop, tile_position=(0, 32 * g),
                )

            for g in range(4):  # ghn: recurrent-only, runs while tanh lands
                for kt in range(4):
                    hT_mm(g, 3, 2, kt, kt == 0, kt == 3)
            for gate in range(2):  # r then z: 4 hT + 2 xp parts
                for g in range(4):
                    for kt in range(4):
                        hT_mm(g, gate, gate, kt, kt == 0, False)
                    for kt in range(2):
                        xp_mm(g, gate, gate, kt, False, kt == 1)
            for g in range(4):  # gxn: input-only
                for kt in range(2):
                    xp_mm(g, 2, 2, kt, kt == 0, kt == 1)

            gb = work.tile([128, 512], F32, tag="gb")
            nc.vector.tensor_add(gb[:], g_ps[:], ball_sb[:])
            rz = work.tile([128, 256], F32, tag="rz")
            nc.scalar.activation(rz[:], gb[:, 0:256], Act.Sigmoid)
            t1 = work.tile([128, 128], F32, tag="t1")
            nc.vector.tensor_mul(t1[:], rz[:, 0:128], gb[:, 384:512])
            t2 = work.tile([128, 128], F32, tag="t2")
            nc.vector.tensor_add(t2[:], t1[:], gb[:, 256:384])
            n_s = work.tile([128, 128], F32, tag="n_s")
            nc.scalar.activation(n_s[:], t2[:], Act.Tanh)

            # h' = n + z*(h-n)
            d_s = work.tile([128, 128], F32, tag="d_s")
            nc.vector.tensor_sub(d_s[:], h_sb[:], n_s[:])
            p_s = work.tile([128, 128], F32, tag="p_s")
            nc.vector.tensor_mul(p_s[:], rz[:, 128:256], d_s[:])
            nc.vector.tensor_add(h_sb[:], n_s[:], p_s[:])

            # hT for next step: one full 128x128 PE transpose
            # (h_sb[32k+b, p] = h[b, 128k+p], so h_sb.T == hT layout)
            hT_ps = tps.tile([128, 128], F32)
            nc.tensor.transpose(hT_ps[:], h_sb[:], ident[:])
            nc.vector.tensor_copy(hT_sb[:], hT_ps[:])

        nc.sync.dma_start(out=hout.ap(), in_=h_sb[:])


_CACHE = {}


def _get_nc():
    if "nc" not in _CACHE:
        _CACHE["nc"] = _build_nc()
    return _CACHE["nc"]


def _prep_shared(W_ih, W_hh, b_ih, b_hh, Wt_h, Wt_x, bt):
    wtx = np.ascontiguousarray(
        np.vstack([Wt_x.T, bt[None, :]]).astype(np.float32)
    )  # [161, 256]
    wthT = np.ascontiguousarray(
        Wt_h.T.reshape(4, 128, XP).transpose(1, 0, 2).reshape(128, 4 * XP)
    ).astype(BF)
    wih = np.ascontiguousarray(
        W_ih.T.reshape(2, 128, GATE3).transpose(1, 0, 2).reshape(128, 2 * GATE3)
    ).astype(BF)
    whh = np.ascontiguousarray(
        W_hh.T.reshape(4, 128, GATE3).transpose(1, 0, 2).reshape(128, 4 * GATE3)
    ).astype(BF)
    b_rz = (b_ih + b_hh)[: 2 * H]
    b_in = b_ih[2 * H :]
    b_hn = b_hh[2 * H :]
    ball = np.zeros((4, 32, 4, 128), np.float32)
    for g in range(4):
        ball[g, :, 0, :] = b_rz[128 * g : 128 * (g + 1)]
        ball[g, :, 1, :] = b_rz[512 + 128 * g : 512 + 128 * (g + 1)]
        ball[g, :, 2, :] = b_in[128 * g : 128 * (g + 1)]
        ball[g, :, 3, :] = b_hn[128 * g : 128 * (g + 1)]
    ball = np.ascontiguousarray(ball.reshape(128, 512))
    return wtx, wthT, wih, whh, ball


def _make_in_maps(x_l_seq, x_t_seq, x_w_seq, x_s_seq, shared):
    wtx, wthT, wih, whh, ball = shared
    x_cat = np.concatenate(
        [np.asarray(x_l_seq), np.asarray(x_t_seq), np.asarray(x_w_seq), np.asarray(x_s_seq)],
        axis=-1,
    ).astype(np.float32)  # [B, T, 160]
    in_maps = []
    for c in range(NCORES):
        xc = x_cat[BC * c : BC * (c + 1)]  # [32, T, 160]
        xTc = xc.transpose(2, 1, 0).reshape(D, T * BC)  # [160, t*32+b]
        xTa = np.vstack([xTc, np.ones((1, T * BC), np.float32)])
        in_maps.append(
            {
                "xT": np.ascontiguousarray(xTa),
                "wtx": wtx,
                "wthT": wthT,
                "wih": wih,
                "whh": whh,
                "ball": ball,
            }
        )
    return in_maps


def kernel(x_l_seq, x_t_seq, x_w_seq, x_s_seq, W_ih, W_hh, b_ih, b_hh, Wt_h, Wt_x, bt):
    nc = _get_nc()
    shared = _prep_shared(
        np.asarray(W_ih, np.float32), np.asarray(W_hh, np.float32),
        np.asarray(b_ih, np.float32), np.asarray(b_hh, np.float32),
        np.asarray(Wt_h, np.float32), np.asarray(Wt_x, np.float32),
        np.asarray(bt, np.float32),
    )
    in_maps = _make_in_maps(x_l_seq, x_t_seq, x_w_seq, x_s_seq, shared)
    res = run_bass_kernel_spmd(nc, in_maps, core_ids=list(range(NCORES)))
    out = np.zeros((1, B, H), np.float32)
    for c in range(NCORES):
        hc = res.results[c]["hout"]  # [128, 128]
        out[0, BC * c : BC * (c + 1), :] = (
            hc.reshape(4, 32, 128).transpose(1, 0, 2).reshape(32, H)
        )
    return out
